# revision 41
# baseline (speedup 1.0000x reference)
"""AttentionBlock (GroupNorm -> QKV 1x1 -> single-head attention -> out proj -> residual)
for x:(4,512,64,64) f32, distributed over 8 NeuronCores.

Sharding: data-parallel over batch, 2 cores per sample, each core owns 2048 of the
4096 query positions. Each core receives a column-ROTATED copy of its sample
(its local 2048 positions first) so the compiled program is identical on every
core (SPMD): Q/residual/output always address columns [0,2048); GroupNorm stats,
K and V use all 4096 columns (both are invariant to the column permutation).

Per-core pipeline:
  1) GroupNorm stats (bn_stats per channel row + PE-transpose trick for the
     cross-partition group reduction) -> per-channel affine (a, b) with
     xn = a*x + b; folded into the projection weights: W' = W diag(a),
     b' = b_w + W b  (so xn is never materialized).
  2) K = W'k x   -> [c, j]  (natural layout, c on partitions)
     V^T = x^T W'v^T -> [j, c]
     Q = W'q x[:, :2048] -> [c, i]
  3) Attention in transposed orientation: E[j,i] = exp(scale * K^T Q) computed
     128 j-rows at a time; denom[i] += ones^T E (PSUM accumulate);
     O[c,i] += V^T[jchunk,:]^T E (PSUM accumulate). No transposes in the loop.
  4) out = Wo (O/denom) + bo + x[:, :2048] -> y [512, 2048].

All matmuls run as float32r (full-rate FP22 multiply, fp32 accumulate).
"""

import sys

sys.path.insert(0, "/opt/trn_rl_repo")

import numpy as np
from contextlib import ExitStack

import concourse.bass as bass
import concourse.tile as tile
from concourse import bacc, mybir
from concourse.masks import make_identity

F32 = mybir.dt.float32
F32R = mybir.dt.float32r

C = 512          # channels
HW = 4096        # spatial positions per sample
L = 2048         # query positions per core
P = 128          # partitions
CO = C // P      # 4 channel chunks
NG = 32          # groups
GS = C // NG     # 16 channels per group
G_PER_CO = P // GS  # 8 groups per 128-partition chunk
EPS = 1e-6
SCALE = C ** -0.5
IB = 256         # query block
NIB = L // IB    # 8
NJ = HW // P     # 32 j-chunks
B = 4            # batch
NCORES = 8

_cached = {}
DEBUG = False


def build_program():
    nc = bacc.Bacc(None, target_bir_lowering=False)

    xf = nc.declare_dram_parameter("xf", [C, HW], F32R, isOutput=False)
    wq_d = nc.declare_dram_parameter("wq", [C, C], F32, isOutput=False)
    wk_d = nc.declare_dram_parameter("wk", [C, C], F32, isOutput=False)
    wv_d = nc.declare_dram_parameter("wv", [C, C], F32, isOutput=False)
    wo_d = nc.declare_dram_parameter("wo", [C, C], F32, isOutput=False)
    bq_d = nc.declare_dram_parameter("bq", [C], F32, isOutput=False)
    bk_d = nc.declare_dram_parameter("bk", [C], F32, isOutput=False)
    bv_d = nc.declare_dram_parameter("bv", [C], F32, isOutput=False)
    bo_d = nc.declare_dram_parameter("bo", [C], F32, isOutput=False)
    gamma_d = nc.declare_dram_parameter("gamma", [C], F32, isOutput=False)
    beta_d = nc.declare_dram_parameter("beta", [C], F32, isOutput=False)
    y = nc.declare_dram_parameter("y", [C, L], F32, isOutput=True)
    if DEBUG:
        dbg_ab = nc.declare_dram_parameter("dbg_ab", [P, 2 * CO], F32, isOutput=True)
        dbg_mv = nc.declare_dram_parameter("dbg_mv", [P, CO, 2], F32, isOutput=True)
        dbg_G = nc.declare_dram_parameter("dbg_G", [8, G_PER_CO], F32, isOutput=True)
        dbg_s = nc.declare_dram_parameter("dbg_s", [CO, 3 * G_PER_CO], F32, isOutput=True)
        dbg_B = nc.declare_dram_parameter("dbg_B", [CO, 2 * P], F32, isOutput=True)
        dbg_K = nc.declare_dram_parameter("dbg_K", [P, CO, HW], F32, isOutput=True)
        dbg_Q = nc.declare_dram_parameter("dbg_Q", [P, CO, L], F32, isOutput=True)

    # [c, j] -> [cp, coo, j] with c = coo*128 + cp
    xf_t = xf[:].rearrange("(coo cp) j -> cp coo j", cp=P)
    y_t = y[:].rearrange("(coo cp) i -> cp coo i", cp=P)

    with tile.TileContext(nc) as tc, ExitStack() as ctx:
        consts = ctx.enter_context(tc.tile_pool(name="consts", bufs=1))
        big = ctx.enter_context(tc.tile_pool(name="big", bufs=1))

        ident = consts.tile([P, P], F32)
        make_identity(nc, ident)
        ones_f32 = consts.tile([P, 1], F32)
        nc.vector.memset(ones_f32, 1.0)
        ones_col = consts.tile([P, 1], F32R)
        nc.vector.tensor_copy(out=ones_col, in_=ones_f32)
        eps_t = consts.tile([CO, 1], F32)
        nc.vector.memset(eps_t, EPS)

        # small per-channel DRAM vectors in [cp, coo] layout
        def load_chan_vec(name, dram):
            t = consts.tile([P, CO], F32, tag=name)
            nc.sync.dma_start(out=t, in_=dram[:].rearrange("(coo cp) -> cp coo", cp=P))
            return t

        gamma_sb = load_chan_vec("gamma_sb", gamma_d)
        beta_sb = load_chan_vec("beta_sb", beta_d)
        bo_sb = load_chan_vec("bo_sb", bo_d)

        def load_row_vec(name, dram):
            t = consts.tile([1, C], F32, tag=name)
            nc.sync.dma_start(out=t, in_=dram[None, :])
            return t

        bq_row = load_row_vec("bq_row", bq_d)
        bk_row = load_row_vec("bk_row", bk_d)
        bv_row = load_row_vec("bv_row", bv_d)
        ones_row_f32 = consts.tile([1, P], F32)
        nc.vector.memset(ones_row_f32, 1.0)
        ones_row = consts.tile([1, P], F32R)
        nc.vector.tensor_copy(out=ones_row, in_=ones_row_f32)
        dram = ctx.enter_context(tc.tile_pool(name="dram", bufs=1, space="DRAM"))

        K_sb = big.tile([P, CO, HW], F32R, tag="K")
        Q_sb = big.tile([P, CO, L], F32R, tag="Q")
        WoT = big.tile([P, CO, C], F32R, tag="WoT")
        # folded biases
        bqf = consts.tile([P, CO], F32, tag="bqf")
        bkf = consts.tile([P, CO], F32, tag="bkf")
        bvf = consts.tile([P, CO], F32, tag="bvf")

        # ---------------- Phase 1: x load, GN stats, weight prep, K/Q proj ----
        with (
            tc.tile_pool(name="ph1", bufs=1) as ph1,
            tc.tile_pool(name="wload", bufs=1) as wload,
            tc.psum_pool(name="pp1", bufs=2) as pp1,
            tc.psum_pool(name="ppb", bufs=2) as ppb,
            tc.psum_pool(name="ppmm", bufs=3) as ppmm,
        ):
            x_sb = ph1.tile([P, CO, HW], F32R, tag="x")
            nc.sync.dma_start(out=x_sb, in_=xf_t)

            # --- GroupNorm statistics ---
            # bn_stats over 512-wide subgroups -> per-channel mean/var
            stats = ph1.tile([P, CO, 8, 6], F32, tag="stats")
            for coo in range(CO):
                xv = x_sb[:, coo, :].rearrange("p (s f) -> p s f", f=512)
                for s in range(8):
                    nc.vector.bn_stats(out=stats[:, coo, s, :], in_=xv[:, s, :])
            mv = ph1.tile([P, CO, 2], F32, tag="mv")
            for coo in range(CO):
                nc.vector.bn_aggr(out=mv[:, coo, :], in_=stats[:, coo, :, :])

            # T_in cols 0:4 = per-channel mean (per coo), cols 4:8 = E[x^2]
            T_in = ph1.tile([P, 8], F32, tag="T_in")
            nc.vector.tensor_copy(T_in[:, 0:CO], mv[:, :, 0])
            nc.vector.tensor_tensor(
                out=T_in[:, CO : 2 * CO], in0=mv[:, :, 0], in1=mv[:, :, 0],
                op=mybir.AluOpType.mult,
            )
            nc.vector.tensor_tensor(
                out=T_in[:, CO : 2 * CO], in0=T_in[:, CO : 2 * CO], in1=mv[:, :, 1],
                op=mybir.AluOpType.add,
            )
            # transpose [128, 8] -> [8, 128]
            tps = pp1.tile([8, P], F32, tag="wtp")
            nc.tensor.transpose(tps, T_in, ident)
            T_sb = ph1.tile([8, P], F32, tag="T_sb")
            nc.vector.tensor_copy(T_sb, tps)
            # group-sum along free dim: [8, 8g, 16] -> [8, 8]
            G = ph1.tile([8, G_PER_CO], F32, tag="G")
            nc.vector.reduce_sum(
                out=G, in_=T_sb.rearrange("p (g s) -> p g s", s=GS),
                axis=mybir.AxisListType.X,
            )
            # move E[x^2] rows (4:8) onto partitions 0:4
            G2 = ph1.tile([CO, G_PER_CO], F32, tag="G2")
            nc.sync.dma_start(out=G2, in_=G[CO : 2 * CO, :])
            mean_g = ph1.tile([CO, G_PER_CO], F32, tag="mean_g")
            nc.scalar.mul(out=mean_g, in_=G[0:CO, :], mul=1.0 / GS)
            var_g = ph1.tile([CO, G_PER_CO], F32, tag="var_g")
            # var = E[x^2] - mean^2
            nc.vector.tensor_tensor(
                out=var_g, in0=mean_g, in1=mean_g, op=mybir.AluOpType.mult
            )
            nc.vector.tensor_scalar(
                out=G2, in0=G2, scalar1=1.0 / GS, scalar2=None,
                op0=mybir.AluOpType.mult,
            )
            nc.vector.tensor_tensor(
                out=var_g, in0=G2, in1=var_g, op=mybir.AluOpType.subtract
            )
            # rstd = 1/sqrt(var + eps)
            rstd_g = ph1.tile([CO, G_PER_CO], F32, tag="rstd_g")
            nc.scalar.activation(
                out=rstd_g, in_=var_g, func=mybir.ActivationFunctionType.Sqrt,
                bias=eps_t, scale=1.0,
            )
            nc.vector.reciprocal(out=rstd_g, in_=rstd_g)

            # broadcast groups -> channels along free dim: [4, 8] -> [4, 128]
            Bm = ph1.tile([CO, P], F32, tag="Bm")
            Br = ph1.tile([CO, P], F32, tag="Br")
            for src, dst in ((mean_g, Bm), (rstd_g, Br)):
                bc = bass.AP(
                    tensor=src.tensor, offset=src.offset,
                    ap=[src.ap[0], src.ap[1], [0, GS]],
                )
                nc.vector.tensor_copy(dst.rearrange("p (g s) -> p g s", s=GS), bc)
            # transpose back to [128, 4]
            mean_ps = pp1.tile([P, CO], F32, tag="wtp", name="mean_ps")
            rstd_ps = pp1.tile([P, CO], F32, tag="wtp", name="rstd_ps")
            nc.tensor.transpose(mean_ps, Bm, ident[0:CO, 0:CO])
            nc.tensor.transpose(rstd_ps, Br, ident[0:CO, 0:CO])
            # a = gamma * rstd ; b = beta - mean * a   (per channel, [128, 4])
            a_ch = consts.tile([P, CO], F32, tag="a_ch")
            b_ch = consts.tile([P, CO], F32R, tag="b_ch")
            nc.vector.tensor_tensor(
                out=a_ch, in0=gamma_sb, in1=rstd_ps, op=mybir.AluOpType.mult
            )
            nc.vector.tensor_tensor(
                out=b_ch, in0=mean_ps, in1=a_ch, op=mybir.AluOpType.mult
            )
            nc.vector.tensor_tensor(
                out=b_ch, in0=beta_sb, in1=b_ch, op=mybir.AluOpType.subtract
            )

            # --- weight transposes WxT[ci_p, cio, co] = W[co, ci] ---
            def transpose_weight(dram, dest):
                wsb = wload.tile([P, CO, C], F32, tag="wsb")
                nc.sync.dma_start(
                    out=wsb, in_=dram[:].rearrange("(coo cp) ci -> cp coo ci", cp=P)
                )
                for coo in range(CO):
                    for cio in range(CO):
                        tp = pp1.tile([P, P], F32, tag="wtp")
                        nc.tensor.transpose(
                            tp, wsb[:, coo, cio * P : (cio + 1) * P], ident
                        )
                        nc.vector.tensor_copy(
                            out=dest[:, cio, coo * P : (coo + 1) * P], in_=tp
                        )

            WqT = ph1.tile([P, CO, C], F32R, tag="WqT")
            WkT = ph1.tile([P, CO, C], F32R, tag="WkT")
            transpose_weight(wq_d, WqT)
            transpose_weight(wk_d, WkT)
            transpose_weight(wo_d, WoT)

            # --- bias folds: b'[co] = b[co] + sum_ci W[co, ci] b_ch[ci], done in
            # row form (M=1 lhsT, N=512 moving — N=1 moving is invalid ISA for
            # f32r), then bounced through DRAM into [cp, coo] channel layout.
            # Uses unscaled WT; Tile orders these before the in-place scaling.
            def fold_bias(WT, brow, bdst, nm, psum_pool, sb_pool):
                pb = psum_pool.tile([1, C], F32, tag="pbias", name=f"pb_{nm}")
                for cio in range(CO):
                    nc.tensor.matmul(
                        pb, lhsT=b_ch[:, cio : cio + 1], rhs=WT[:, cio, :],
                        start=(cio == 0), stop=(cio == CO - 1),
                    )
                brow_sum = sb_pool.tile([1, C], F32, tag="brow_sum", name=f"bs_{nm}")
                nc.vector.tensor_tensor(
                    out=brow_sum, in0=pb, in1=brow, op=mybir.AluOpType.add
                )
                scr = dram.tile([1, C], F32, name=f"scr_{nm}")
                nc.sync.dma_start(out=scr, in_=brow_sum)
                nc.sync.dma_start(
                    out=bdst, in_=scr[0, :].rearrange("(coo cp) -> cp coo", cp=P)
                )

            fold_bias(WqT, bq_row, bqf, "q", ppb, ph1)
            fold_bias(WkT, bk_row, bkf, "k", ppb, ph1)

            # scale WqT, WkT in place: W'T[ci, co] = WT[ci, co] * a[ci]
            for WT in (WqT, WkT):
                for cio in range(CO):
                    nc.vector.tensor_scalar_mul(
                        WT[:, cio, :], WT[:, cio, :], a_ch[:, cio : cio + 1]
                    )

            if DEBUG:
                nc.sync.dma_start(out=dbg_s[:][:, 0:8], in_=mean_g)
                nc.sync.dma_start(out=dbg_s[:][:, 8:16], in_=var_g)
                nc.sync.dma_start(out=dbg_s[:][:, 16:24], in_=rstd_g)
                nc.sync.dma_start(out=dbg_B[:][:, 0:P], in_=Bm)
                nc.sync.dma_start(out=dbg_B[:][:, P : 2 * P], in_=Br)
                nc.sync.dma_start(out=dbg_ab[:][:, 0:CO], in_=a_ch)
                nc.sync.dma_start(out=dbg_ab[:][:, CO : 2 * CO], in_=b_ch)
                nc.sync.dma_start(out=dbg_mv[:], in_=mv)
                nc.sync.dma_start(out=dbg_G[:], in_=G)

            # --- K projection: K[co, j] ---
            for jc in range(HW // 512):
                for coo in range(CO):
                    pk = ppmm.tile([P, 512], F32, tag="pk")
                    for cio in range(CO):
                        nc.tensor.matmul(
                            pk, lhsT=WkT[:, cio, coo * P : (coo + 1) * P],
                            rhs=x_sb[:, cio, jc * 512 : (jc + 1) * 512],
                            start=(cio == 0), stop=(cio == CO - 1),
                        )
                    nc.vector.tensor_scalar(
                        out=K_sb[:, coo, jc * 512 : (jc + 1) * 512], in0=pk,
                        scalar1=bkf[:, coo : coo + 1], scalar2=None,
                        op0=mybir.AluOpType.add,
                    )

            # --- Q projection: Q[co, i], i in local half = cols [0, 2048) ---
            for ic in range(L // 512):
                for coo in range(CO):
                    pq = ppmm.tile([P, 512], F32, tag="pk")
                    for cio in range(CO):
                        nc.tensor.matmul(
                            pq, lhsT=WqT[:, cio, coo * P : (coo + 1) * P],
                            rhs=x_sb[:, cio, ic * 512 : (ic + 1) * 512],
                            start=(cio == 0), stop=(cio == CO - 1),
                        )
                    nc.vector.tensor_scalar(
                        out=Q_sb[:, coo, ic * 512 : (ic + 1) * 512], in0=pq,
                        scalar1=bqf[:, coo : coo + 1], scalar2=None,
                        op0=mybir.AluOpType.add,
                    )

            if DEBUG:
                nc.sync.dma_start(out=dbg_K[:], in_=K_sb)
                nc.sync.dma_start(out=dbg_Q[:], in_=Q_sb)

        # ---------------- Phase 2: V^T projection (x re-streamed) -------------
        vtp = ctx.enter_context(tc.tile_pool(name="vtp", bufs=1))
        VT_sb = vtp.tile([P, NJ, C], F32R, tag="VT")
        with (
            tc.tile_pool(name="ph2", bufs=1) as ph2,
            tc.tile_pool(name="xchunk", bufs=4) as xchunk,
            tc.psum_pool(name="pp2", bufs=3) as pp2,
        ):
            # Prime fresh PSUM slots with DVE memsets so cross-phase pool-reuse
            # waits land on DVE (no wait-count limit) instead of matmul LW
            # instructions (max 4 sync waits in S3_LW).
            def prime_psum(pool, shape, tag, n):
                for i in range(n):
                    t = pool.tile(shape, F32, tag=tag, name=f"prime_{tag}{i}")
                    nc.vector.memset(t, 0.0)

            WvT = ph2.tile([P, CO, C], F32R, tag="WvT")
            prime_psum(pp2, [P, C], "pv", 3)
            with tc.tile_pool(name="wload2", bufs=1) as wload2, tc.psum_pool(
                name="ppt2", bufs=2
            ) as ppt2:
                prime_psum(ppt2, [P, P], "wtp2", 2)
                prime_psum(ppt2, [1, C], "pbv", 2)
                wsb = wload2.tile([P, CO, C], F32, tag="wsb2")
                nc.sync.dma_start(
                    out=wsb, in_=wv_d[:].rearrange("(coo cp) ci -> cp coo ci", cp=P)
                )
                # Collapse the DMA-queue waits onto the DVE clock so the first
                # transpose matmul stays within the 4-sync-wait LW limit.
                nc.vector.tensor_copy(out=wsb, in_=wsb)
                for coo in range(CO):
                    for cio in range(CO):
                        tp = ppt2.tile([P, P], F32, tag="wtp2")
                        nc.tensor.transpose(
                            tp, wsb[:, coo, cio * P : (cio + 1) * P], ident
                        )
                        nc.vector.tensor_copy(
                            out=WvT[:, cio, coo * P : (coo + 1) * P], in_=tp
                        )
                # bvf[c] = bv[c] + sum_ci Wv[c, ci] * b[ci]  (channel layout,
                # applied after attention: O = O0/denom + bvf since
                # sum_j (A[i,j]) = 1 after normalization)
                pbv = ppt2.tile([1, C], F32, tag="pbv")
                for cio in range(CO):
                    nc.tensor.matmul(
                        pbv, lhsT=b_ch[:, cio : cio + 1], rhs=WvT[:, cio, :],
                        start=(cio == 0), stop=(cio == CO - 1),
                    )
                bvrow = ph2.tile([1, C], F32, tag="bvrow")
                nc.vector.tensor_tensor(
                    out=bvrow, in0=pbv, in1=bv_row, op=mybir.AluOpType.add
                )
                scrv = dram.tile([1, C], F32, name="scr_v")
                nc.sync.dma_start(out=scrv, in_=bvrow)
                nc.sync.dma_start(
                    out=bvf, in_=scrv[0, :].rearrange("(coo cp) -> cp coo", cp=P)
                )
                # scale WvT in place
                for cio in range(CO):
                    nc.vector.tensor_scalar_mul(
                        WvT[:, cio, :], WvT[:, cio, :], a_ch[:, cio : cio + 1]
                    )

            for jc in range(NJ):
                xc = xchunk.tile([P, CO, P], F32R, tag="xc")
                nc.sync.dma_start(out=xc, in_=xf_t[:, :, jc * P : (jc + 1) * P])
                pv = pp2.tile([P, C], F32, tag="pv")
                for cio in range(CO):
                    nc.tensor.matmul(
                        pv, lhsT=xc[:, cio, :], rhs=WvT[:, cio, :],
                        start=(cio == 0), stop=(cio == CO - 1),
                    )
                nc.vector.tensor_copy(out=VT_sb[:, jc, :], in_=pv)

        # ---------------- Phase 3: attention + output projection --------------
        with (
            tc.tile_pool(name="att", bufs=2) as att,
            tc.tile_pool(name="esb", bufs=3) as esb,
            tc.psum_pool(name="pe", bufs=2) as pe,
            tc.psum_pool(name="po", bufs=4) as po,
            tc.psum_pool(name="pd", bufs=2) as pd,
        ):
            def prime_psum3(pool, shape, tag, n):
                for i in range(n):
                    t = pool.tile(shape, F32, tag=tag, name=f"prime3_{tag}{i}")
                    nc.vector.memset(t, 0.0)

            prime_psum3(pe, [P, IB], "eps", 2)
            prime_psum3(po, [P, IB], "ops", 4)
            prime_psum3(pd, [1, IB], "dps", 2)
            for ib in range(NIB):
                isl = slice(ib * IB, (ib + 1) * IB)
                dps = pd.tile([1, IB], F32, tag="dps")
                ops = [
                    po.tile([P, IB], F32, tag="ops", name=f"ops{i}")
                    for i in range(CO)
                ]
                for jc in range(NJ):
                    eps_ps = pe.tile([P, IB], F32, tag="eps")
                    for cio in range(CO):
                        nc.tensor.matmul(
                            eps_ps,
                            lhsT=K_sb[:, cio, jc * P : (jc + 1) * P],
                            rhs=Q_sb[:, cio, isl],
                            start=(cio == 0), stop=(cio == CO - 1),
                        )
                    e_sb = esb.tile([P, IB], F32R, tag="e_sb")
                    nc.scalar.activation(
                        out=e_sb, in_=eps_ps,
                        func=mybir.ActivationFunctionType.Exp, scale=SCALE,
                    )
                    nc.tensor.matmul(
                        dps, lhsT=ones_col, rhs=e_sb,
                        start=(jc == 0), stop=(jc == NJ - 1),
                    )
                    for cio in range(CO):
                        nc.tensor.matmul(
                            ops[cio],
                            lhsT=VT_sb[:, jc, cio * P : (cio + 1) * P],
                            rhs=e_sb,
                            start=(jc == 0), stop=(jc == NJ - 1),
                        )
                recip = att.tile([1, IB], F32R, tag="recip")
                with nc.allow_low_precision(reason="f32r holds full fp32 bits"):
                    nc.vector.reciprocal(out=recip, in_=dps)
                # broadcast 1/denom across partitions via K=1 outer product
                bcast_ps = pe.tile([P, IB], F32, tag="eps", name="bcast_ps")
                nc.tensor.matmul(
                    bcast_ps, lhsT=ones_row, rhs=recip,
                    start=True, stop=True,
                )
                bcast_sb = esb.tile([P, IB], F32, tag="e_sb", name="bcast_sb")
                nc.vector.tensor_copy(out=bcast_sb, in_=bcast_ps)
                O_sb = att.tile([P, CO, IB], F32R, tag="O_sb")
                for cio in range(CO):
                    nc.vector.tensor_tensor(
                        out=O_sb[:, cio, :], in0=ops[cio], in1=bcast_sb,
                        op=mybir.AluOpType.mult,
                    )
                    nc.vector.tensor_scalar(
                        out=O_sb[:, cio, :], in0=O_sb[:, cio, :],
                        scalar1=bvf[:, cio : cio + 1], scalar2=None,
                        op0=mybir.AluOpType.add,
                    )
                xres = att.tile([P, CO, IB], F32, tag="xres")
                nc.gpsimd.dma_start(out=xres, in_=xf_t[:, :, isl])
                for coo in range(CO):
                    fps = pe.tile([P, IB], F32, tag="eps")
                    for cio in range(CO):
                        nc.tensor.matmul(
                            fps, lhsT=WoT[:, cio, coo * P : (coo + 1) * P],
                            rhs=O_sb[:, cio, :],
                            start=(cio == 0), stop=(cio == CO - 1),
                        )
                    nc.vector.tensor_tensor(
                        out=xres[:, coo, :], in0=fps, in1=xres[:, coo, :],
                        op=mybir.AluOpType.add,
                    )
                    nc.vector.tensor_scalar(
                        out=xres[:, coo, :], in0=xres[:, coo, :],
                        scalar1=bo_sb[:, coo : coo + 1], scalar2=None,
                        op0=mybir.AluOpType.add,
                    )
                nc.sync.dma_start(out=y_t[:, :, isl], in_=xres)

    nc.compile()
    return nc


def get_program():
    if "nc" not in _cached:
        _cached["nc"] = build_program()
    return _cached["nc"]


def make_in_maps(inputs):
    x = np.asarray(inputs["x"], np.float32).reshape(B, C, HW)
    common = {
        k: np.ascontiguousarray(np.asarray(inputs[k], np.float32))
        for k in ("wq", "wk", "wv", "wo", "bq", "bk", "bv", "bo", "gamma", "beta")
    }
    in_maps = []
    for core in range(NCORES):
        b, h = core // 2, core % 2
        loc = x[b][:, h * L : (h + 1) * L]
        oth = x[b][:, (1 - h) * L : (2 - h) * L]
        xf_rot = np.ascontiguousarray(np.concatenate([loc, oth], axis=1))
        m = dict(common)
        m["xf"] = xf_rot
        in_maps.append(m)
    return in_maps


def kernel(**inputs) -> np.ndarray:
    from concourse.bass_utils import run_bass_kernel_spmd

    nc = get_program()
    in_maps = make_in_maps(inputs)
    res = run_bass_kernel_spmd(nc, in_maps, list(range(NCORES)))
    out = np.empty((B, C, HW), np.float32)
    for core in range(NCORES):
        b, h = core // 2, core % 2
        out[b][:, h * L : (h + 1) * L] = res.results[core]["y"]
    return out.reshape(B, C, 64, 64)
